# revision 11
# baseline (speedup 1.0000x reference)
"""Kalman filter kernel for 8x Trainium2 NeuronCores.

Math: the covariance/gain recursion (P_t, K_t) is data-independent and
converges to steady state within ~30 steps.  After convergence the state
recursion is the LTI scan  z_t = M z_{t-1} + NK @ [u_t; x_t]  with
M = (I-KC)A (spectral radius ~0.50),  NK = [(I-KC)B, K].  ||M^8|| ~ 3e-3,
so against the 2e-2 gate the scan truncates to an 8-tap causal FIR,
factored as two stages:

    g(t) = NK v(t) + M^4 NK v(t-4)            (2 taps, dilation 4, K=128)
    z(t) = sum_{r<4} M^r g(t-r)               (4 taps, dilation 1)

Stage 2's four K=64 taps are packed into two K=128 matmuls by stacking
[g(t); g(t-1)] on partitions (two shifted copies of stage 1's PSUM
output), so each core runs just 8 bf16 matmuls over its 1024 columns.
All matmuls are bf16 (fp32 runs 2-pass LOW/HIGH at 1/4 rate); the output
DMAs straight from PSUM.  Host adds two fp32 corrections: the transient
patch for t<96 (time-varying gains) and the 3 leading columns of cores
1..7 (left-halo taps the device reads as zeros).
"""

import numpy as np
import ml_dtypes

L = 64          # latent size
NV = 128        # stacked input dim [u; x]
T = 8192
NCORES = 8
TC = T // NCORES            # 1024 output columns per core
HALO = 8                    # left v-halo per core (stage1 reads back 4+stage2 3)
WIDTH = HALO + TC           # per-core input columns (1032)
S1TAPS = 2                  # stage-1 taps, dilation 4
S1DIL = 4
S2TAPS = 4                  # stage-2 taps, dilation 1 (packed 2x K=128)
NRIC = 64                   # Riccati iterations (converged far past f32 by then)
T0 = 96                     # transient patch columns
CHUNK = 512                 # PSUM bank = 512 fp32 columns

F32 = np.float32
BF16 = ml_dtypes.bfloat16


# ----------------------------------------------------------------------------
# host-side parameter preprocessing (data-independent)
# ----------------------------------------------------------------------------

def _gains(A, B, C, Q, R):
    """float64 Riccati recursion -> per-step (M_t, NK_t) lists."""
    Ad, Bd, Cd, Qd, Rd = (m.astype(np.float64) for m in (A, B, C, Q, R))
    P = np.eye(L)
    Ms, NKs = [], []
    for _ in range(NRIC):
        Pp = Ad @ P @ Ad.T + Qd
        S = Cd @ Pp @ Cd.T + Rd
        K = Pp @ Cd.T @ np.linalg.inv(S)
        P = Pp - K @ (Cd @ Pp)
        IKC = np.eye(L) - K @ Cd
        Ms.append(IKC @ Ad)
        NKs.append(np.concatenate([IKC @ Bd, K], axis=1))   # [L, NV]
    return Ms, NKs


def _mpow(M, k):
    out = np.eye(L)
    for _ in range(k):
        out = M @ out
    return out


def _bf(x):
    return np.asarray(x, F32).astype(BF16).astype(F32)


def _weights(Mss, NKss):
    """bf16 tap matrices.  w1[p] = M^(4p) NK  (stage 1, [L,NV]);
    w2[r] = M^r (stage 2, [L,L]).  Returned as f32 arrays holding exact
    bf16 values (shared by device upload and host replica)."""
    w1 = [_bf(_mpow(Mss, S1DIL * p) @ NKss) for p in range(S1TAPS)]
    w2 = [_bf(_mpow(Mss, r)) for r in range(S2TAPS)]
    return w1, w2


def _stage1_host(w1, vq, cols):
    """g at the given global columns (list), replicating the device:
    bf16 inputs/weights, fp32 accumulate.  vq: f32-holding-bf16 [NV,T]."""
    out = np.zeros((L, len(cols)), F32)
    for j, c in enumerate(cols):
        acc = np.zeros(L, F32)
        for p in range(S1TAPS):
            cc = c - S1DIL * p
            if cc >= 0:
                acc += w1[p] @ vq[:, cc]
        out[:, j] = acc
    return out


def _fir_host(w1, w2, vq, ncols):
    """Device-pipeline replica for global cols [0, ncols): zero left pad,
    bf16 rounding of g between stages."""
    pad = S1DIL * (S1TAPS - 1) + S2TAPS  # enough left context
    vp = np.concatenate([np.zeros((NV, pad), F32), vq[:, :ncols]], axis=1)
    n = vp.shape[1]
    g = np.zeros((L, n), F32)
    for p in range(S1TAPS):
        sh = S1DIL * p
        g[:, sh:] += (w1[p] @ vp[:, : n - sh]).astype(F32)
    gq = _bf(g)
    gq[:, :pad] = 0.0  # device sees zeros left of its first column
    z = np.zeros((L, n), F32)
    for r in range(S2TAPS):
        z[:, r:] += (w2[r] @ gq[:, : n - r]).astype(F32)
    return z[:, pad:]


def _transient_patch(v, vq, Ms, NKs, w1, w2):
    """Additive correction for cols [0,T0): exact time-varying recursion
    minus the device FIR replica."""
    z = np.zeros(L, F32)
    z_exact = np.zeros((L, T0), F32)
    for t in range(T0):
        Mt = (Ms[t] if t < NRIC else Ms[-1]).astype(F32)
        NKt = (NKs[t] if t < NRIC else NKs[-1]).astype(F32)
        z = Mt @ z + NKt @ v[:, t]
        z_exact[:, t] = z
    return z_exact - _fir_host(w1, w2, vq, T0)


# ----------------------------------------------------------------------------
# device kernel
# ----------------------------------------------------------------------------

_CACHE = {}


def _build_nc():
    import concourse.mybir as mybir
    from concourse import bacc
    from concourse.tile import TileContext

    f32 = mybir.dt.float32
    bf16 = mybir.dt.bfloat16
    nc = bacc.Bacc()

    # w (4 lhsT slots, 256 cols) and v (WIDTH cols) combined in one dram
    # tensor so a single SWDGE dma covers [w | first v chunk].
    WCOLS = (S1TAPS + 2) * L
    vw_d = nc.dram_tensor("vw", [NV, WCOLS + WIDTH], bf16, kind="ExternalInput")
    z_d = nc.dram_tensor("z", [L, TC], bf16, kind="ExternalOutput")

    chunks = [(HALO + i * CHUNK, HALO + (i + 1) * CHUNK) for i in range(TC // CHUNK)]
    NWARM = 4

    with TileContext(nc) as tc:
        with (
            tc.tile_pool(name="sb", bufs=1) as sb,
            tc.tile_pool(name="ps1", bufs=2, space="PSUM") as ps1,
            tc.tile_pool(name="ps2", bufs=2, space="PSUM") as ps2,
            tc.tile_pool(name="psw", bufs=1, space="PSUM") as psw,
        ):
            # Every DMA queue is descriptor-rate-bound (~36-53 packets/us,
            # one packet per partition row), and the three queues (sync-HW,
            # scalar-HW, gpsimd-SW) sustain that rate concurrently.  So
            # stripe each transfer across the queues BY PARTITION (keeping
            # full-size rows): 128 rows -> ~43 per queue -> ~1.2us.
            vw_sb = sb.tile([NV, WCOLS + WIDTH], bf16)
            w_sb = vw_sb[:, 0:WCOLS]
            v_sb = vw_sb[:, WCOLS : WCOLS + WIDTH]
            QS = (nc.sync, nc.scalar, nc.gpsimd)
            PCUT = (0, 43, 86, NV)  # partition stripes
            CCUT = (0, WCOLS + 520, WCOLS + WIDTH)  # column groups
            for ci in range(2):
                for qi, q in enumerate(QS):
                    q.dma_start(
                        out=vw_sb[
                            PCUT[qi] : PCUT[qi + 1], CCUT[ci] : CCUT[ci + 1]
                        ],
                        in_=vw_d[
                            PCUT[qi] : PCUT[qi + 1], CCUT[ci] : CCUT[ci + 1]
                        ],
                    )

            def wslot(i):  # lhsT slot i: [NV, L]
                return w_sb[:, i * L : (i + 1) * L]

            # PE p-state ramps to 2.4 GHz only after ~3us of continuous
            # work; burn the input-DMA wait warming it on a zeroed tile.
            scratch = sb.tile([NV, CHUNK], bf16)
            nc.vector.memset(scratch, 0.0)
            wacc = psw.tile([L, CHUNK], f32)
            for _ in range(NWARM):
                nc.tensor.matmul(
                    out=wacc, lhsT=scratch[:, 0:L], rhs=scratch,
                    start=True, stop=True,
                )
            zcut = (0, 22, 43, L)  # output partition stripes

            # stacked stage-1 output: partitions 0-63 g(t), 64-127 g(t-1)
            gs = sb.tile([NV, WIDTH + 2], bf16)
            nc.vector.memset(gs[0:L, 0:HALO], 0.0)
            nc.vector.memset(gs[L:NV, 0 : HALO + 1], 0.0)

            z_sb = sb.tile([L, TC], bf16)
            for ci, (lo, hi) in enumerate(chunks):
                acc = ps1.tile([L, CHUNK], f32)
                for p in range(S1TAPS):
                    nc.tensor.matmul(
                        out=acc,
                        lhsT=wslot(p),
                        rhs=v_sb[:, lo - S1DIL * p : hi - S1DIL * p],
                        start=(p == 0),
                        stop=(p == S1TAPS - 1),
                    )
                nc.vector.tensor_copy(out=gs[0:L, lo:hi], in_=acc)
                nc.vector.tensor_copy(out=gs[L:NV, lo + 1 : hi + 1], in_=acc)

                acc2 = ps2.tile([L, CHUNK], f32)
                nc.tensor.matmul(
                    out=acc2, lhsT=wslot(S1TAPS), rhs=gs[:, lo:hi],
                    start=True, stop=False,
                )
                nc.tensor.matmul(
                    out=acc2, lhsT=wslot(S1TAPS + 1), rhs=gs[:, lo - 2 : hi - 2],
                    start=False, stop=True,
                )
                nc.scalar.copy(out=z_sb[:, lo - HALO : hi - HALO], in_=acc2)
                for qi, q in enumerate(QS):
                    q.dma_start(
                        out=z_d[zcut[qi] : zcut[qi + 1], lo - HALO : hi - HALO],
                        in_=z_sb[zcut[qi] : zcut[qi + 1], lo - HALO : hi - HALO],
                    )

    nc.compile()
    return nc


def _prep(inputs, controls, A, B, C, Q, R):
    """Host preprocessing shared by kernel() and the profiling path.
    Returns (in_maps, patch, bfixes) where patch is the [L,T0] transient
    correction and bfixes[i] the [L,3] left-halo fix for core i>=1."""
    v = np.concatenate(
        [np.ascontiguousarray(controls, F32), np.ascontiguousarray(inputs, F32)],
        axis=0,
    )  # [NV, T]
    vq = _bf(v)

    Ms, NKs = _gains(A, B, C, Q, R)
    w1, w2 = _weights(Ms[-1], NKs[-1])
    patch = _transient_patch(v, vq, Ms, NKs, w1, w2)

    # device weight block: 4 lhsT slots [NV, L] side by side -> [NV, 4L]
    wslots = [w.T for w in w1]  # [NV, L] each
    wslots.append(np.concatenate([w2[0].T, w2[1].T], axis=0))  # [NV, L]
    wslots.append(np.concatenate([w2[2].T, w2[3].T], axis=0))
    w_dev = np.concatenate(wslots, axis=1)  # [NV, 4L]

    vpad = np.concatenate([np.zeros((NV, HALO), F32), vq], axis=1)
    in_maps = [
        {
            "vw": np.ascontiguousarray(
                np.concatenate(
                    [w_dev, vpad[:, i * TC : i * TC + WIDTH]], axis=1
                )
            ).astype(BF16),
        }
        for i in range(NCORES)
    ]

    # left-halo fixes: device g is zero for local cols < HALO, i.e. global
    # cols < i*TC; output col j in {0,1,2} of core i>=1 is missing
    # sum_{r>j} w2[r] g(i*TC + j - r).
    bfixes = {}
    for i in range(1, NCORES):
        gcols = [i * TC - 3, i * TC - 2, i * TC - 1]
        gh = _bf(_stage1_host(w1, vq, gcols))  # [L,3] bf16-rounded like device
        fix = np.zeros((L, 3), F32)
        for j in range(3):
            for r in range(j + 1, S2TAPS):
                fix[:, j] += w2[r] @ gh[:, 3 + j - r]
        bfixes[i] = fix
    return in_maps, patch, bfixes


def kernel(inputs, controls, A, B, C, Q, R):
    from concourse.bass_utils import run_bass_kernel_spmd

    in_maps, patch, bfixes = _prep(inputs, controls, A, B, C, Q, R)

    if "nc" not in _CACHE:
        _CACHE["nc"] = _build_nc()
    res = run_bass_kernel_spmd(_CACHE["nc"], in_maps, core_ids=list(range(NCORES)))

    z = np.concatenate(
        [np.asarray(res.results[i]["z"]).astype(F32) for i in range(NCORES)], axis=1
    )
    z[:, :T0] += patch
    for i, fix in bfixes.items():
        z[:, i * TC : i * TC + 3] += fix
    return z


# revision 16
# speedup vs baseline: 1.4538x; 1.4538x over previous
"""Kalman filter kernel for 8x Trainium2 NeuronCores.

Math: the covariance/gain recursion (P_t, K_t) is data-independent and
converges to steady state within ~30 steps.  After convergence the state
recursion is the LTI scan  z_t = M z_{t-1} + NK @ [u_t; x_t]  with
M = (I-KC)A (spectral radius ~0.50),  NK = [(I-KC)B, K].  ||M^8|| ~ 3e-3,
so against the 2e-2 gate the scan truncates to an 8-tap causal FIR,
factored as two stages:

    g(t) = NK v(t) + M^4 NK v(t-4)            (2 taps, dilation 4, K=128)
    z(t) = sum_{r<4} M^r g(t-r)               (4 taps, dilation 1)

Stage 2's four K=64 taps are packed into two K=128 matmuls by stacking
[g(t); g(t-1)] on partitions (two shifted copies of stage 1's PSUM
output), so each core runs just 8 bf16 matmuls over its 1024 columns.
All matmuls are bf16 (fp32 runs 2-pass LOW/HIGH at 1/4 rate); the output
DMAs straight from PSUM.  Host adds two fp32 corrections: the transient
patch for t<96 (time-varying gains) and the 3 leading columns of cores
1..7 (left-halo taps the device reads as zeros).
"""

import numpy as np
import ml_dtypes

L = 64          # latent size
NV = 128        # stacked input dim [u; x]
T = 8192
NCORES = 8
TC = T // NCORES            # 1024 output columns per core
HALO = 8                    # left v-halo per core (stage1 reads back 4+stage2 3)
WIDTH = HALO + TC           # per-core input columns (1032)
S1TAPS = 2                  # stage-1 taps, dilation 4
S1DIL = 4
S2TAPS = 4                  # stage-2 taps, dilation 1 (packed 2x K=128)
NRIC = 64                   # Riccati iterations (converged far past f32 by then)
T0 = 96                     # transient patch columns
CHUNK = 512                 # PSUM bank = 512 fp32 columns

F32 = np.float32
BF16 = ml_dtypes.bfloat16


# ----------------------------------------------------------------------------
# host-side parameter preprocessing (data-independent)
# ----------------------------------------------------------------------------

def _gains(A, B, C, Q, R):
    """float64 Riccati recursion -> per-step (M_t, NK_t) lists."""
    Ad, Bd, Cd, Qd, Rd = (m.astype(np.float64) for m in (A, B, C, Q, R))
    P = np.eye(L)
    Ms, NKs = [], []
    for _ in range(NRIC):
        Pp = Ad @ P @ Ad.T + Qd
        S = Cd @ Pp @ Cd.T + Rd
        K = Pp @ Cd.T @ np.linalg.inv(S)
        P = Pp - K @ (Cd @ Pp)
        IKC = np.eye(L) - K @ Cd
        Ms.append(IKC @ Ad)
        NKs.append(np.concatenate([IKC @ Bd, K], axis=1))   # [L, NV]
    return Ms, NKs


def _mpow(M, k):
    out = np.eye(L)
    for _ in range(k):
        out = M @ out
    return out


def _bf(x):
    return np.asarray(x, F32).astype(BF16).astype(F32)


def _weights(Mss, NKss):
    """bf16 tap matrices.  w1[p] = M^(4p) NK  (stage 1, [L,NV]);
    w2[r] = M^r (stage 2, [L,L]).  Returned as f32 arrays holding exact
    bf16 values (shared by device upload and host replica)."""
    w1 = [_bf(_mpow(Mss, S1DIL * p) @ NKss) for p in range(S1TAPS)]
    w2 = [_bf(_mpow(Mss, r)) for r in range(S2TAPS)]
    return w1, w2


def _stage1_host(w1, vq, cols):
    """g at the given global columns (list), replicating the device:
    bf16 inputs/weights, fp32 accumulate.  vq: f32-holding-bf16 [NV,T]."""
    out = np.zeros((L, len(cols)), F32)
    for j, c in enumerate(cols):
        acc = np.zeros(L, F32)
        for p in range(S1TAPS):
            cc = c - S1DIL * p
            if cc >= 0:
                acc += w1[p] @ vq[:, cc]
        out[:, j] = acc
    return out


def _fir_host(w1, w2, vq, ncols):
    """Device-pipeline replica for global cols [0, ncols): zero left pad,
    bf16 rounding of g between stages."""
    pad = S1DIL * (S1TAPS - 1) + S2TAPS  # enough left context
    vp = np.concatenate([np.zeros((NV, pad), F32), vq[:, :ncols]], axis=1)
    n = vp.shape[1]
    g = np.zeros((L, n), F32)
    for p in range(S1TAPS):
        sh = S1DIL * p
        g[:, sh:] += (w1[p] @ vp[:, : n - sh]).astype(F32)
    gq = _bf(g)
    gq[:, :pad] = 0.0  # device sees zeros left of its first column
    z = np.zeros((L, n), F32)
    for r in range(S2TAPS):
        z[:, r:] += (w2[r] @ gq[:, : n - r]).astype(F32)
    return z[:, pad:]


def _transient_patch(v, vq, Ms, NKs, w1, w2):
    """Additive correction for cols [0,T0): exact time-varying recursion
    minus the device FIR replica."""
    z = np.zeros(L, F32)
    z_exact = np.zeros((L, T0), F32)
    for t in range(T0):
        Mt = (Ms[t] if t < NRIC else Ms[-1]).astype(F32)
        NKt = (NKs[t] if t < NRIC else NKs[-1]).astype(F32)
        z = Mt @ z + NKt @ v[:, t]
        z_exact[:, t] = z
    return z_exact - _fir_host(w1, w2, vq, T0)


# ----------------------------------------------------------------------------
# device kernel
# ----------------------------------------------------------------------------

_CACHE = {}


def _build_nc():
    import concourse.mybir as mybir
    from concourse import bacc
    from concourse.tile import TileContext

    f32 = mybir.dt.float32
    bf16 = mybir.dt.bfloat16
    nc = bacc.Bacc()

    # w (4 lhsT slots, 256 cols) and v (WIDTH cols) combined in one dram
    # tensor so a single SWDGE dma covers [w | first v chunk].
    WCOLS = (S1TAPS + 2) * L
    vw_d = nc.dram_tensor("vw", [NV, WCOLS + WIDTH], bf16, kind="ExternalInput")
    z_d = nc.dram_tensor("z", [L, TC], bf16, kind="ExternalOutput")

    chunks = [(HALO + i * CHUNK, HALO + (i + 1) * CHUNK) for i in range(TC // CHUNK)]
    NWARM = 5

    with TileContext(nc) as tc:
        with (
            tc.tile_pool(name="sb", bufs=1) as sb,
            tc.tile_pool(name="ps1", bufs=2, space="PSUM") as ps1,
            tc.tile_pool(name="ps2", bufs=2, space="PSUM") as ps2,
            tc.tile_pool(name="psw", bufs=1, space="PSUM") as psw,
        ):
            # gpsimd's software DGE aggregates rows into ~3-5KB packets
            # (few packets -> fast; every queue is packet-rate-bound), so it
            # carries the critical first transfer ALONE.  The second v chunk
            # rides the two hardware-DGE queues, split by partition so each
            # moves only 64 rows.
            vw_sb = sb.tile([NV, WCOLS + WIDTH], bf16)
            w_sb = vw_sb[:, 0:WCOLS]
            v_sb = vw_sb[:, WCOLS : WCOLS + WIDTH]
            CMID = WCOLS + 520
            nc.gpsimd.dma_start(out=vw_sb[:, 0:CMID], in_=vw_d[:, 0:CMID])
            nc.sync.dma_start(out=vw_sb[0:L, CMID:], in_=vw_d[0:L, CMID:])
            nc.scalar.dma_start(out=vw_sb[L:NV, CMID:], in_=vw_d[L:NV, CMID:])

            def wslot(i):  # lhsT slot i: [NV, L]
                return w_sb[:, i * L : (i + 1) * L]

            # PE p-state ramps to 2.4 GHz only after ~3us of continuous
            # work; burn the input-DMA wait warming it on a zeroed tile.
            scratch = sb.tile([NV, CHUNK], bf16)
            nc.vector.memset(scratch, 0.0)
            wacc = psw.tile([L, CHUNK], f32)
            for _ in range(NWARM):
                nc.tensor.matmul(
                    out=wacc, lhsT=scratch[:, 0:L], rhs=scratch,
                    start=True, stop=True,
                )

            # stacked stage-1 output: partitions 0-63 g(t), 64-127 g(t-1)
            gs = sb.tile([NV, WIDTH + 2], bf16)
            nc.vector.memset(gs[0:L, 0:HALO], 0.0)
            nc.vector.memset(gs[L:NV, 0 : HALO + 1], 0.0)

            z_sb = sb.tile([L, TC], bf16)
            for ci, (lo, hi) in enumerate(chunks):
                acc = ps1.tile([L, CHUNK], f32)
                for p in range(S1TAPS):
                    nc.tensor.matmul(
                        out=acc,
                        lhsT=wslot(p),
                        rhs=v_sb[:, lo - S1DIL * p : hi - S1DIL * p],
                        start=(p == 0),
                        stop=(p == S1TAPS - 1),
                    )
                nc.vector.tensor_copy(out=gs[0:L, lo:hi], in_=acc)
                nc.scalar.copy(out=gs[L:NV, lo + 1 : hi + 1], in_=acc)

                acc2 = ps2.tile([L, CHUNK], f32)
                nc.tensor.matmul(
                    out=acc2, lhsT=wslot(S1TAPS), rhs=gs[:, lo:hi],
                    start=True, stop=False,
                )
                nc.tensor.matmul(
                    out=acc2, lhsT=wslot(S1TAPS + 1), rhs=gs[:, lo - 2 : hi - 2],
                    start=False, stop=True,
                )
                nc.vector.tensor_copy(out=z_sb[:, lo - HALO : hi - HALO], in_=acc2)
                nc.gpsimd.dma_start(
                    out=z_d[:, lo - HALO : hi - HALO],
                    in_=z_sb[:, lo - HALO : hi - HALO],
                )

    nc.compile()
    return nc


def _prep(inputs, controls, A, B, C, Q, R):
    """Host preprocessing shared by kernel() and the profiling path.
    Returns (in_maps, patch, bfixes) where patch is the [L,T0] transient
    correction and bfixes[i] the [L,3] left-halo fix for core i>=1."""
    v = np.concatenate(
        [np.ascontiguousarray(controls, F32), np.ascontiguousarray(inputs, F32)],
        axis=0,
    )  # [NV, T]
    vq = _bf(v)

    Ms, NKs = _gains(A, B, C, Q, R)
    w1, w2 = _weights(Ms[-1], NKs[-1])
    patch = _transient_patch(v, vq, Ms, NKs, w1, w2)

    # device weight block: 4 lhsT slots [NV, L] side by side -> [NV, 4L]
    wslots = [w.T for w in w1]  # [NV, L] each
    wslots.append(np.concatenate([w2[0].T, w2[1].T], axis=0))  # [NV, L]
    wslots.append(np.concatenate([w2[2].T, w2[3].T], axis=0))
    w_dev = np.concatenate(wslots, axis=1)  # [NV, 4L]

    vpad = np.concatenate([np.zeros((NV, HALO), F32), vq], axis=1)
    in_maps = [
        {
            "vw": np.ascontiguousarray(
                np.concatenate(
                    [w_dev, vpad[:, i * TC : i * TC + WIDTH]], axis=1
                )
            ).astype(BF16),
        }
        for i in range(NCORES)
    ]

    # left-halo fixes: device g is zero for local cols < HALO, i.e. global
    # cols < i*TC; output col j in {0,1,2} of core i>=1 is missing
    # sum_{r>j} w2[r] g(i*TC + j - r).
    bfixes = {}
    for i in range(1, NCORES):
        gcols = [i * TC - 3, i * TC - 2, i * TC - 1]
        gh = _bf(_stage1_host(w1, vq, gcols))  # [L,3] bf16-rounded like device
        fix = np.zeros((L, 3), F32)
        for j in range(3):
            for r in range(j + 1, S2TAPS):
                fix[:, j] += w2[r] @ gh[:, 3 + j - r]
        bfixes[i] = fix
    return in_maps, patch, bfixes


def kernel(inputs, controls, A, B, C, Q, R):
    from concourse.bass_utils import run_bass_kernel_spmd

    in_maps, patch, bfixes = _prep(inputs, controls, A, B, C, Q, R)

    if "nc" not in _CACHE:
        _CACHE["nc"] = _build_nc()
    res = run_bass_kernel_spmd(_CACHE["nc"], in_maps, core_ids=list(range(NCORES)))

    z = np.concatenate(
        [np.asarray(res.results[i]["z"]).astype(F32) for i in range(NCORES)], axis=1
    )
    z[:, :T0] += patch
    for i, fix in bfixes.items():
        z[:, i * TC : i * TC + 3] += fix
    return z


# revision 20
# speedup vs baseline: 1.4673x; 1.0093x over previous
"""Kalman filter kernel for 8x Trainium2 NeuronCores.

Math: the covariance/gain recursion (P_t, K_t) is data-independent and
converges to steady state within ~30 steps.  After convergence the state
recursion is the LTI scan  z_t = M z_{t-1} + NK @ [u_t; x_t]  with
M = (I-KC)A (spectral radius ~0.50),  NK = [(I-KC)B, K].  ||M^8|| ~ 3e-3,
so against the 2e-2 gate the scan truncates to an 8-tap causal FIR,
factored as two stages:

    g(t) = NK v(t) + M^4 NK v(t-4)            (2 taps, dilation 4, K=128)
    z(t) = sum_{r<4} M^r g(t-r)               (4 taps, dilation 1)

Stage 2's four K=64 taps are packed into two K=128 matmuls by stacking
[g(t); g(t-1)] on partitions (two shifted copies of stage 1's PSUM
output), so each core runs just 8 bf16 matmuls over its 1024 columns.
All matmuls are bf16 (fp32 runs 2-pass LOW/HIGH at 1/4 rate); the output
DMAs straight from PSUM.  Host adds two fp32 corrections: the transient
patch for t<96 (time-varying gains) and the 3 leading columns of cores
1..7 (left-halo taps the device reads as zeros).
"""

import numpy as np
import ml_dtypes

L = 64          # latent size
NV = 128        # stacked input dim [u; x]
T = 8192
NCORES = 8
TC = T // NCORES            # 1024 output columns per core
HALO = 8                    # left v-halo per core (stage1 reads back 4+stage2 3)
WIDTH = HALO + TC           # per-core input columns (1032)
S1TAPS = 2                  # stage-1 taps, dilation 4
S1DIL = 4
S2TAPS = 4                  # stage-2 taps, dilation 1 (packed 2x K=128)
NRIC = 64                   # Riccati iterations (converged far past f32 by then)
T0 = 96                     # transient patch columns
CHUNK = 512                 # PSUM bank = 512 fp32 columns

F32 = np.float32
BF16 = ml_dtypes.bfloat16


# ----------------------------------------------------------------------------
# host-side parameter preprocessing (data-independent)
# ----------------------------------------------------------------------------

def _gains(A, B, C, Q, R):
    """float64 Riccati recursion -> per-step (M_t, NK_t) lists."""
    Ad, Bd, Cd, Qd, Rd = (m.astype(np.float64) for m in (A, B, C, Q, R))
    P = np.eye(L)
    Ms, NKs = [], []
    for _ in range(NRIC):
        Pp = Ad @ P @ Ad.T + Qd
        S = Cd @ Pp @ Cd.T + Rd
        K = Pp @ Cd.T @ np.linalg.inv(S)
        P = Pp - K @ (Cd @ Pp)
        IKC = np.eye(L) - K @ Cd
        Ms.append(IKC @ Ad)
        NKs.append(np.concatenate([IKC @ Bd, K], axis=1))   # [L, NV]
    return Ms, NKs


def _mpow(M, k):
    out = np.eye(L)
    for _ in range(k):
        out = M @ out
    return out


def _bf(x):
    return np.asarray(x, F32).astype(BF16).astype(F32)


def _weights(Mss, NKss):
    """bf16 tap matrices.  w1[p] = M^(4p) NK  (stage 1, [L,NV]);
    w2[r] = M^r (stage 2, [L,L]).  Returned as f32 arrays holding exact
    bf16 values (shared by device upload and host replica)."""
    w1 = [_bf(_mpow(Mss, S1DIL * p) @ NKss) for p in range(S1TAPS)]
    w2 = [_bf(_mpow(Mss, r)) for r in range(S2TAPS)]
    return w1, w2


def _stage1_host(w1, vq, cols):
    """g at the given global columns (list), replicating the device:
    bf16 inputs/weights, fp32 accumulate.  vq: f32-holding-bf16 [NV,T]."""
    out = np.zeros((L, len(cols)), F32)
    for j, c in enumerate(cols):
        acc = np.zeros(L, F32)
        for p in range(S1TAPS):
            cc = c - S1DIL * p
            if cc >= 0:
                acc += w1[p] @ vq[:, cc]
        out[:, j] = acc
    return out


def _fir_host(w1, w2, vq, ncols):
    """Device-pipeline replica for global cols [0, ncols): zero left pad,
    bf16 rounding of g between stages."""
    pad = S1DIL * (S1TAPS - 1) + S2TAPS  # enough left context
    vp = np.concatenate([np.zeros((NV, pad), F32), vq[:, :ncols]], axis=1)
    n = vp.shape[1]
    g = np.zeros((L, n), F32)
    for p in range(S1TAPS):
        sh = S1DIL * p
        g[:, sh:] += (w1[p] @ vp[:, : n - sh]).astype(F32)
    gq = _bf(g)
    gq[:, :pad] = 0.0  # device sees zeros left of its first column
    z = np.zeros((L, n), F32)
    for r in range(S2TAPS):
        z[:, r:] += (w2[r] @ gq[:, : n - r]).astype(F32)
    return z[:, pad:]


def _transient_patch(v, vq, Ms, NKs, w1, w2):
    """Additive correction for cols [0,T0): exact time-varying recursion
    minus the device FIR replica."""
    z = np.zeros(L, F32)
    z_exact = np.zeros((L, T0), F32)
    for t in range(T0):
        Mt = (Ms[t] if t < NRIC else Ms[-1]).astype(F32)
        NKt = (NKs[t] if t < NRIC else NKs[-1]).astype(F32)
        z = Mt @ z + NKt @ v[:, t]
        z_exact[:, t] = z
    return z_exact - _fir_host(w1, w2, vq, T0)


# ----------------------------------------------------------------------------
# device kernel
# ----------------------------------------------------------------------------

_CACHE = {}


def _build_nc():
    import concourse.mybir as mybir
    from concourse import bacc
    from concourse.tile import TileContext

    f32 = mybir.dt.float32
    bf16 = mybir.dt.bfloat16
    nc = bacc.Bacc()

    # w (4 lhsT slots, 256 cols) and v (WIDTH cols) combined in one dram
    # tensor so a single SWDGE dma covers [w | first v chunk].
    WCOLS = (S1TAPS + 2) * L
    vw_d = nc.dram_tensor("vw", [NV, WCOLS + WIDTH], bf16, kind="ExternalInput")
    z_d = nc.dram_tensor("z", [L, TC], bf16, kind="ExternalOutput")

    chunks = [(HALO + i * CHUNK, HALO + (i + 1) * CHUNK) for i in range(TC // CHUNK)]
    NWARM = 7

    with TileContext(nc) as tc:
        with (
            tc.tile_pool(name="sb", bufs=1) as sb,
            tc.tile_pool(name="ps1", bufs=2, space="PSUM") as ps1,
            tc.tile_pool(name="ps2", bufs=2, space="PSUM") as ps2,
            tc.tile_pool(name="psw", bufs=1, space="PSUM") as psw,
        ):
            # Both hardware-DGE queues (sync, scalar) stripe every transfer
            # by partition halves: 64 full-width rows per queue move at
            # ~120 packets/us when the streams are this small.  gpsimd's
            # SWDGE is avoided entirely -- its first use pays a ~1.2us
            # ucode warmup that would land on the critical path.
            vw_sb = sb.tile([NV, WCOLS + WIDTH], bf16)
            w_sb = vw_sb[:, 0:WCOLS]
            v_sb = vw_sb[:, WCOLS : WCOLS + WIDTH]
            CMID = WCOLS + 520
            nc.sync.dma_start(out=vw_sb[0:L, 0:CMID], in_=vw_d[0:L, 0:CMID])
            nc.scalar.dma_start(out=vw_sb[L:NV, 0:CMID], in_=vw_d[L:NV, 0:CMID])
            nc.sync.dma_start(out=vw_sb[0:L, CMID:], in_=vw_d[0:L, CMID:])
            nc.scalar.dma_start(out=vw_sb[L:NV, CMID:], in_=vw_d[L:NV, CMID:])

            def wslot(i):  # lhsT slot i: [NV, L]
                return w_sb[:, i * L : (i + 1) * L]

            # PE p-state ramps to 2.4 GHz only after ~3us of continuous
            # work; burn the input-DMA wait warming it on a zeroed tile.
            scratch = sb.tile([NV, CHUNK], bf16)
            nc.vector.memset(scratch, 0.0)
            wacc = psw.tile([L, CHUNK], f32)
            for _ in range(NWARM):
                nc.tensor.matmul(
                    out=wacc, lhsT=scratch[:, 0:L], rhs=scratch,
                    start=True, stop=True,
                )

            # stacked stage-1 output: partitions 0-63 g(t), 64-127 g(t-1)
            gs = sb.tile([NV, WIDTH + 2], bf16)
            nc.vector.memset(gs[0:L, 0:HALO], 0.0)
            nc.vector.memset(gs[L:NV, 0 : HALO + 1], 0.0)

            z_sb = sb.tile([L, TC], bf16)
            for ci, (lo, hi) in enumerate(chunks):
                acc = ps1.tile([L, CHUNK], f32)
                for p in range(S1TAPS):
                    nc.tensor.matmul(
                        out=acc,
                        lhsT=wslot(p),
                        rhs=v_sb[:, lo - S1DIL * p : hi - S1DIL * p],
                        start=(p == 0),
                        stop=(p == S1TAPS - 1),
                    )
                nc.scalar.copy(out=gs[L:NV, lo + 1 : hi + 1], in_=acc)
                nc.vector.tensor_copy(out=gs[0:L, lo:hi], in_=acc)

                acc2 = ps2.tile([L, CHUNK], f32)
                nc.tensor.matmul(
                    out=acc2, lhsT=wslot(S1TAPS), rhs=gs[:, lo:hi],
                    start=True, stop=False,
                )
                nc.tensor.matmul(
                    out=acc2, lhsT=wslot(S1TAPS + 1), rhs=gs[:, lo - 2 : hi - 2],
                    start=False, stop=True,
                )
                nc.vector.tensor_copy(out=z_sb[:, lo - HALO : hi - HALO], in_=acc2)
                nc.sync.dma_start(
                    out=z_d[0:32, lo - HALO : hi - HALO],
                    in_=z_sb[0:32, lo - HALO : hi - HALO],
                )
                nc.scalar.dma_start(
                    out=z_d[32:L, lo - HALO : hi - HALO],
                    in_=z_sb[32:L, lo - HALO : hi - HALO],
                )

    nc.compile()
    return nc


def _prep(inputs, controls, A, B, C, Q, R):
    """Host preprocessing shared by kernel() and the profiling path.
    Returns (in_maps, patch, bfixes) where patch is the [L,T0] transient
    correction and bfixes[i] the [L,3] left-halo fix for core i>=1."""
    v = np.concatenate(
        [np.ascontiguousarray(controls, F32), np.ascontiguousarray(inputs, F32)],
        axis=0,
    )  # [NV, T]
    vq = _bf(v)

    Ms, NKs = _gains(A, B, C, Q, R)
    w1, w2 = _weights(Ms[-1], NKs[-1])
    patch = _transient_patch(v, vq, Ms, NKs, w1, w2)

    # device weight block: 4 lhsT slots [NV, L] side by side -> [NV, 4L]
    wslots = [w.T for w in w1]  # [NV, L] each
    wslots.append(np.concatenate([w2[0].T, w2[1].T], axis=0))  # [NV, L]
    wslots.append(np.concatenate([w2[2].T, w2[3].T], axis=0))
    w_dev = np.concatenate(wslots, axis=1)  # [NV, 4L]

    vpad = np.concatenate([np.zeros((NV, HALO), F32), vq], axis=1)
    in_maps = [
        {
            "vw": np.ascontiguousarray(
                np.concatenate(
                    [w_dev, vpad[:, i * TC : i * TC + WIDTH]], axis=1
                )
            ).astype(BF16),
        }
        for i in range(NCORES)
    ]

    # left-halo fixes: device g is zero for local cols < HALO, i.e. global
    # cols < i*TC; output col j in {0,1,2} of core i>=1 is missing
    # sum_{r>j} w2[r] g(i*TC + j - r).
    bfixes = {}
    for i in range(1, NCORES):
        gcols = [i * TC - 3, i * TC - 2, i * TC - 1]
        gh = _bf(_stage1_host(w1, vq, gcols))  # [L,3] bf16-rounded like device
        fix = np.zeros((L, 3), F32)
        for j in range(3):
            for r in range(j + 1, S2TAPS):
                fix[:, j] += w2[r] @ gh[:, 3 + j - r]
        bfixes[i] = fix
    return in_maps, patch, bfixes


def kernel(inputs, controls, A, B, C, Q, R):
    from concourse.bass_utils import run_bass_kernel_spmd

    in_maps, patch, bfixes = _prep(inputs, controls, A, B, C, Q, R)

    if "nc" not in _CACHE:
        _CACHE["nc"] = _build_nc()
    res = run_bass_kernel_spmd(_CACHE["nc"], in_maps, core_ids=list(range(NCORES)))

    z = np.concatenate(
        [np.asarray(res.results[i]["z"]).astype(F32) for i in range(NCORES)], axis=1
    )
    z[:, :T0] += patch
    for i, fix in bfixes.items():
        z[:, i * TC : i * TC + 3] += fix
    return z


# revision 23
# speedup vs baseline: 1.4770x; 1.0066x over previous
"""Kalman filter kernel for 8x Trainium2 NeuronCores.

Math: the covariance/gain recursion (P_t, K_t) is data-independent and
converges to steady state within ~30 steps.  After convergence the state
recursion is the LTI scan  z_t = M z_{t-1} + NK @ [u_t; x_t]  with
M = (I-KC)A (spectral radius ~0.50),  NK = [(I-KC)B, K].  ||M^8|| ~ 3e-3,
so against the 2e-2 gate the scan truncates to an 8-tap causal FIR,
factored as two stages:

    g(t) = NK v(t) + M^4 NK v(t-4)            (2 taps, dilation 4, K=128)
    z(t) = sum_{r<4} M^r g(t-r)               (4 taps, dilation 1)

Stage 2's four K=64 taps are packed into two K=128 matmuls by stacking
[g(t); g(t-1)] on partitions (two shifted copies of stage 1's PSUM
output), so each core runs just 8 bf16 matmuls over its 1024 columns.
All matmuls are bf16 (fp32 runs 2-pass LOW/HIGH at 1/4 rate); the output
DMAs straight from PSUM.  Host adds two fp32 corrections: the transient
patch for t<96 (time-varying gains) and the 3 leading columns of cores
1..7 (left-halo taps the device reads as zeros).
"""

import numpy as np
import ml_dtypes

L = 64          # latent size
NV = 128        # stacked input dim [u; x]
T = 8192
NCORES = 8
TC = T // NCORES            # 1024 output columns per core
HALO = 8                    # left v-halo per core (stage1 reads back 4+stage2 3)
WIDTH = HALO + TC           # per-core input columns (1032)
S1TAPS = 2                  # stage-1 taps, dilation 4
S1DIL = 4
S2TAPS = 4                  # stage-2 taps, dilation 1 (packed 2x K=128)
NRIC = 64                   # Riccati iterations (converged far past f32 by then)
T0 = 96                     # transient patch columns
CHUNK = 512                 # PSUM bank = 512 fp32 columns

F32 = np.float32
BF16 = ml_dtypes.bfloat16


# ----------------------------------------------------------------------------
# host-side parameter preprocessing (data-independent)
# ----------------------------------------------------------------------------

def _gains(A, B, C, Q, R):
    """float64 Riccati recursion -> per-step (M_t, NK_t) lists."""
    Ad, Bd, Cd, Qd, Rd = (m.astype(np.float64) for m in (A, B, C, Q, R))
    P = np.eye(L)
    Ms, NKs = [], []
    for _ in range(NRIC):
        Pp = Ad @ P @ Ad.T + Qd
        S = Cd @ Pp @ Cd.T + Rd
        K = Pp @ Cd.T @ np.linalg.inv(S)
        P = Pp - K @ (Cd @ Pp)
        IKC = np.eye(L) - K @ Cd
        Ms.append(IKC @ Ad)
        NKs.append(np.concatenate([IKC @ Bd, K], axis=1))   # [L, NV]
    return Ms, NKs


def _mpow(M, k):
    out = np.eye(L)
    for _ in range(k):
        out = M @ out
    return out


def _bf(x):
    return np.asarray(x, F32).astype(BF16).astype(F32)


def _weights(Mss, NKss):
    """bf16 tap matrices.  w1[p] = M^(4p) NK  (stage 1, [L,NV]);
    w2[r] = M^r (stage 2, [L,L]).  Returned as f32 arrays holding exact
    bf16 values (shared by device upload and host replica)."""
    w1 = [_bf(_mpow(Mss, S1DIL * p) @ NKss) for p in range(S1TAPS)]
    w2 = [_bf(_mpow(Mss, r)) for r in range(S2TAPS)]
    return w1, w2


def _stage1_host(w1, vq, cols):
    """g at the given global columns (list), replicating the device:
    bf16 inputs/weights, fp32 accumulate.  vq: f32-holding-bf16 [NV,T]."""
    out = np.zeros((L, len(cols)), F32)
    for j, c in enumerate(cols):
        acc = np.zeros(L, F32)
        for p in range(S1TAPS):
            cc = c - S1DIL * p
            if cc >= 0:
                acc += w1[p] @ vq[:, cc]
        out[:, j] = acc
    return out


def _fir_host(w1, w2, vq, ncols):
    """Device-pipeline replica for global cols [0, ncols): zero left pad,
    bf16 rounding of g between stages."""
    pad = S1DIL * (S1TAPS - 1) + S2TAPS  # enough left context
    vp = np.concatenate([np.zeros((NV, pad), F32), vq[:, :ncols]], axis=1)
    n = vp.shape[1]
    g = np.zeros((L, n), F32)
    for p in range(S1TAPS):
        sh = S1DIL * p
        g[:, sh:] += (w1[p] @ vp[:, : n - sh]).astype(F32)
    gq = _bf(g)
    gq[:, :pad] = 0.0  # device sees zeros left of its first column
    z = np.zeros((L, n), F32)
    for r in range(S2TAPS):
        z[:, r:] += (w2[r] @ gq[:, : n - r]).astype(F32)
    return z[:, pad:]


def _transient_patch(v, vq, Ms, NKs, w1, w2):
    """Additive correction for cols [0,T0): exact time-varying recursion
    minus the device FIR replica."""
    z = np.zeros(L, F32)
    z_exact = np.zeros((L, T0), F32)
    for t in range(T0):
        Mt = (Ms[t] if t < NRIC else Ms[-1]).astype(F32)
        NKt = (NKs[t] if t < NRIC else NKs[-1]).astype(F32)
        z = Mt @ z + NKt @ v[:, t]
        z_exact[:, t] = z
    return z_exact - _fir_host(w1, w2, vq, T0)


# ----------------------------------------------------------------------------
# device kernel
# ----------------------------------------------------------------------------

_CACHE = {}


def _build_nc():
    import concourse.mybir as mybir
    from concourse import bacc
    from concourse.tile import TileContext

    f32 = mybir.dt.float32
    bf16 = mybir.dt.bfloat16
    nc = bacc.Bacc()

    # w (4 lhsT slots, 256 cols) and v (WIDTH cols) combined in one dram
    # tensor so a single SWDGE dma covers [w | first v chunk].
    WCOLS = (S1TAPS + 2) * L
    vw_d = nc.dram_tensor("vw", [NV, WCOLS + WIDTH], bf16, kind="ExternalInput")
    z_d = nc.dram_tensor("z", [L, TC], bf16, kind="ExternalOutput")

    chunks = [(HALO + i * CHUNK, HALO + (i + 1) * CHUNK) for i in range(TC // CHUNK)]
    NWARM = 7

    with TileContext(nc) as tc:
        with (
            tc.tile_pool(name="sb", bufs=1) as sb,
            tc.tile_pool(name="ps1", bufs=2, space="PSUM") as ps1,
            tc.tile_pool(name="ps2", bufs=2, space="PSUM") as ps2,
            tc.tile_pool(name="psw", bufs=1, space="PSUM") as psw,
        ):
            # Both hardware-DGE queues (sync, scalar) stripe every transfer
            # by partition halves: 64 full-width rows per queue move at
            # ~120 packets/us when the streams are this small.  gpsimd's
            # SWDGE is avoided entirely -- its first use pays a ~1.2us
            # ucode warmup that would land on the critical path.
            vw_sb = sb.tile([NV, WCOLS + WIDTH], bf16)
            w_sb = vw_sb[:, 0:WCOLS]
            v_sb = vw_sb[:, WCOLS : WCOLS + WIDTH]
            CMID = WCOLS + 520
            nc.gpsimd.dma_start(out=vw_sb[:, 0:CMID], in_=vw_d[:, 0:CMID])
            nc.sync.dma_start(out=vw_sb[0:L, CMID:], in_=vw_d[0:L, CMID:])
            nc.scalar.dma_start(out=vw_sb[L:NV, CMID:], in_=vw_d[L:NV, CMID:])

            def wslot(i):  # lhsT slot i: [NV, L]
                return w_sb[:, i * L : (i + 1) * L]

            # PE p-state ramps to 2.4 GHz only after ~3us of continuous
            # work; burn the input-DMA wait warming it on a zeroed tile.
            scratch = sb.tile([NV, CHUNK], bf16)
            nc.vector.memset(scratch, 0.0)
            wacc = psw.tile([L, CHUNK], f32)
            for _ in range(NWARM):
                nc.tensor.matmul(
                    out=wacc, lhsT=scratch[:, 0:L], rhs=scratch,
                    start=True, stop=True,
                )

            # stacked stage-1 output: partitions 0-63 g(t), 64-127 g(t-1)
            gs = sb.tile([NV, WIDTH + 2], bf16)
            nc.vector.memset(gs[0:L, 0:HALO], 0.0)
            nc.vector.memset(gs[L:NV, 0 : HALO + 1], 0.0)

            z_sb = sb.tile([L, TC], bf16)
            for ci, (lo, hi) in enumerate(chunks):
                acc = ps1.tile([L, CHUNK], f32)
                for p in range(S1TAPS):
                    nc.tensor.matmul(
                        out=acc,
                        lhsT=wslot(p),
                        rhs=v_sb[:, lo - S1DIL * p : hi - S1DIL * p],
                        start=(p == 0),
                        stop=(p == S1TAPS - 1),
                    )
                # Tile serializes same-PSUM-bank accesses, so the two
                # shifted copies of one chunk can't run concurrently on
                # two engines; instead chunk 0's pair goes to ACT and
                # chunk 1's to DVE so the PAIRS overlap across chunks.
                ceng = nc.scalar.copy if ci == 0 else nc.vector.tensor_copy
                ceng(out=gs[L:NV, lo + 1 : hi + 1], in_=acc)
                ceng(out=gs[0:L, lo:hi], in_=acc)

                acc2 = ps2.tile([L, CHUNK], f32)
                nc.tensor.matmul(
                    out=acc2, lhsT=wslot(S1TAPS), rhs=gs[:, lo:hi],
                    start=True, stop=False,
                )
                nc.tensor.matmul(
                    out=acc2, lhsT=wslot(S1TAPS + 1), rhs=gs[:, lo - 2 : hi - 2],
                    start=False, stop=True,
                )
                zeng = nc.scalar.copy if ci == 0 else nc.vector.tensor_copy
                zeng(out=z_sb[:, lo - HALO : hi - HALO], in_=acc2)
                nc.sync.dma_start(
                    out=z_d[0:32, lo - HALO : hi - HALO],
                    in_=z_sb[0:32, lo - HALO : hi - HALO],
                )
                nc.scalar.dma_start(
                    out=z_d[32:L, lo - HALO : hi - HALO],
                    in_=z_sb[32:L, lo - HALO : hi - HALO],
                )

    nc.compile()
    return nc


def _prep(inputs, controls, A, B, C, Q, R):
    """Host preprocessing shared by kernel() and the profiling path.
    Returns (in_maps, patch, bfixes) where patch is the [L,T0] transient
    correction and bfixes[i] the [L,3] left-halo fix for core i>=1."""
    v = np.concatenate(
        [np.ascontiguousarray(controls, F32), np.ascontiguousarray(inputs, F32)],
        axis=0,
    )  # [NV, T]
    vq = _bf(v)

    Ms, NKs = _gains(A, B, C, Q, R)
    w1, w2 = _weights(Ms[-1], NKs[-1])
    patch = _transient_patch(v, vq, Ms, NKs, w1, w2)

    # device weight block: 4 lhsT slots [NV, L] side by side -> [NV, 4L]
    wslots = [w.T for w in w1]  # [NV, L] each
    wslots.append(np.concatenate([w2[0].T, w2[1].T], axis=0))  # [NV, L]
    wslots.append(np.concatenate([w2[2].T, w2[3].T], axis=0))
    w_dev = np.concatenate(wslots, axis=1)  # [NV, 4L]

    vpad = np.concatenate([np.zeros((NV, HALO), F32), vq], axis=1)
    in_maps = [
        {
            "vw": np.ascontiguousarray(
                np.concatenate(
                    [w_dev, vpad[:, i * TC : i * TC + WIDTH]], axis=1
                )
            ).astype(BF16),
        }
        for i in range(NCORES)
    ]

    # left-halo fixes: device g is zero for local cols < HALO, i.e. global
    # cols < i*TC; output col j in {0,1,2} of core i>=1 is missing
    # sum_{r>j} w2[r] g(i*TC + j - r).
    bfixes = {}
    for i in range(1, NCORES):
        gcols = [i * TC - 3, i * TC - 2, i * TC - 1]
        gh = _bf(_stage1_host(w1, vq, gcols))  # [L,3] bf16-rounded like device
        fix = np.zeros((L, 3), F32)
        for j in range(3):
            for r in range(j + 1, S2TAPS):
                fix[:, j] += w2[r] @ gh[:, 3 + j - r]
        bfixes[i] = fix
    return in_maps, patch, bfixes


def kernel(inputs, controls, A, B, C, Q, R):
    from concourse.bass_utils import run_bass_kernel_spmd

    in_maps, patch, bfixes = _prep(inputs, controls, A, B, C, Q, R)

    if "nc" not in _CACHE:
        _CACHE["nc"] = _build_nc()
    res = run_bass_kernel_spmd(_CACHE["nc"], in_maps, core_ids=list(range(NCORES)))

    z = np.concatenate(
        [np.asarray(res.results[i]["z"]).astype(F32) for i in range(NCORES)], axis=1
    )
    z[:, :T0] += patch
    for i, fix in bfixes.items():
        z[:, i * TC : i * TC + 3] += fix
    return z
